# revision 11
# baseline (speedup 1.0000x reference)
"""Trainium2 Bass kernel for a delayed-synaptic layer.

Computes, for full inputs
    buf        [B=32, D=51, P=1024]  (circular delay buffer)
    weight     [P, N=1024]
    delay_raw  [P, N]
the output
    I_syn[b, n] = sum_p w[p,n] * ((1-a)*buf[b, df, p] + a*buf[b, df+1, p])
with x = 50*sigmoid(delay_raw), df = floor(x), a = x - df.

Algorithm: summation-by-parts over the hat expansion.  With
hat_d(x) = relu(1-|x-d|) one has s = sum_d buf_d * hat_d(x) and
hat_d = R_{d-1} - 2 R_d + R_{d+1} for R_e(x) = relu(x - e), so

    I = buf_0^T @ W + (buf_1 - buf_0)^T @ U + sum_{e>=1} c_e^T @ (W * R_e)

with U = x*w and c_e = buf_{e-1} - 2 buf_e + buf_{e+1} (the e = -1, 0 terms
are affine since x > 0).  Each R_e mask needs ONE relu pass:
  * AD2 route: DVE tensor_scalar relu (4x fp16) + DVE tensor_mul (2x fp16)
  * AR  route: ACT Relu(50*sig - e)           + DVE tensor_mul (2x fp16)
(The Pool engine does NO elementwise work: it shares SBUF ports with the DVE
and roughly halves DVE throughput while active.)  All data fp16 (downcast on
host); PSUM accumulates fp32.  PSUM banks are zeroed by an early DVE memset
so every matmul carries start=False and the PE queue (strictly in-order) can
be sequenced purely by estimated operand readiness.

Column pruning: core k owns p in [128k, 128k+128).  For its column n,
R_e[:, n] == 0 for all e >= max_p x[p,n] + margin =: h_n.  The host sorts
columns by h_n descending (per-core permutation of weight/delay/output
columns) so step e only needs the first m_e = #{n : h_n > e} columns, a
~1.7x cut in mask + matmul work; excluded columns compute exactly zero.
m_e (max across cores) and the data-derived e-range are baked into the
compiled program; different inputs simply recompile.

Wide-m e's run first: the right PSUM bank (columns 512+) then closes well
before the end, and its copy-out + DMA overlap the remaining small-m work.
Input DMAs use whole tensors (2 KB/partition descriptors) on three
independent DGE queues: BUF via Pool SWDGE, delays via SP, weights via ACT.

Sharding: data-parallel over pre-neurons p (contraction axis); each core
emits a partial [32, 1024] output (own column order); host un-permutes+sums.
"""

import numpy as np

B = 32
D_FULL = 51
P = 1024
N = 1024
N_CORES = 8
P_SH = P // N_CORES  # 128
XMARGIN = 0.25  # host-window safety vs device fp16/table sigmoid

_PROGRAM_CACHE: dict = {}
_LAST_PLAN: dict = {}

# clean-rate estimates (ns/col, measured on HW) -- scheduling only
_TS = 0.33  # DVE tensor_scalar fp16 (4x mode)
_TT = 0.59  # DVE tensor_tensor fp16 (2x mode)
_AC = 1.01  # ACT activation pass
_OH_D, _OH_A = 60.0, 90.0


def _plan(delay_raw):
    """Host planning: e-range, per-core column permutations, shared m_e."""
    x = 50.0 / (1.0 + np.exp(-delay_raw.astype(np.float64)))  # [P, N]
    h_all = x.max() + XMARGIN
    e_hi = max(1, int(np.floor(h_all)))  # R_e == 0 for e >= h_all
    e_hi = min(e_hi, 49)
    perms = []
    m_pc = np.zeros((N_CORES, e_hi + 1), dtype=np.int64)
    for k in range(N_CORES):
        h = x[k * P_SH : (k + 1) * P_SH].max(axis=0) + XMARGIN  # [N]
        perm = np.argsort(-h, kind="stable")
        perms.append(perm)
        hs = h[perm]
        for e in range(1, e_hi + 1):
            m_pc[k, e] = int(np.count_nonzero(hs > e))
    m = m_pc.max(axis=0)
    m_list = []
    for e in range(1, e_hi + 1):
        if m[e] <= 0:
            break
        m_list.append(int(min(N, ((m[e] + 63) // 64) * 64)))
    if not m_list:
        m_list = [64]
    return tuple(m_list), perms


def _sched(m_list):
    """Greedy DVE/ACT route balance + PE order by estimated readiness.
    Processing order is e ascending (widest masks first) so the right PSUM
    bank closes early.  Returns (routes, pe_order) with pseudo-items
    "WL","WR","UL","UR" included in pe_order."""
    E = len(m_list)
    # relative ns offsets: DVE mask stream starts ~0 (X50 done), ACT relus
    # start ~+200 (SIG done); U done ~+700 on the DVE queue.
    tD, tA = 0.0, 600.0
    ready = {}
    routes = {}
    for i in range(E):
        m = m_list[i]
        if i < 2:  # DVE-only starters while ACT finishes the sigmoid
            routes[i] = "AD2"
            tD = tD + (_TS + _TT) * m + 2 * _OH_D
            ready[i] = tD
            continue
        cD = tD + (_TS + _TT) * m + 2 * _OH_D
        relu_done = tA + _AC * m + _OH_A
        cA = max(relu_done, tD) + _TT * m + _OH_D
        if cD <= cA:
            routes[i] = "AD2"
            tD = cD
            ready[i] = cD
        else:
            routes[i] = "AR"
            tA = relu_done
            tD = max(tD, relu_done) + _TT * m + _OH_D
            ready[i] = tD
    ready["WL"], ready["WR"] = -200.0, -100.0
    u_done = min(ready[i] for i in range(min(2, E))) if E else 0.0
    ready["UL"], ready["UR"] = u_done + 600.0, u_done + 610.0
    for z in range(4):  # PE warm-up fillers (zero-accumulate) in early gaps
        ready[f"Z{z}"] = -150.0 + 500.0 * z
    pe_order = sorted(ready, key=lambda k: (ready[k], str(k)))
    return routes, pe_order


def _build_program(cfg):
    """Build the (SPMD, identical-per-core) Bass program once per config."""
    from contextlib import ExitStack

    import concourse.tile as tile
    from concourse import bacc, mybir

    m_list = list(cfg)
    E = len(m_list)  # masks for e = 1..E
    D_HI = E + 2  # buf slices used: [0, D_HI); c_e needs buf_{e+1}

    f32 = mybir.dt.float32
    f16 = mybir.dt.float16
    i32 = mybir.dt.int32
    AF = mybir.ActivationFunctionType
    OP = mybir.AluOpType

    routes, pe_order = _sched(m_list)
    _LAST_PLAN.update(m_list=m_list, routes=routes, pe_order=pe_order)

    nc = bacc.Bacc(trn_type="TRN2", target_bir_lowering=False, debug=False)

    dr_d = nc.dram_tensor("delay_sh", [P_SH, N], f16, kind="ExternalInput").ap()
    w_d = nc.dram_tensor("weight_sh", [P_SH, N], f16, kind="ExternalInput").ap()
    buf_d = nc.dram_tensor("buf_sh", [P_SH, D_HI * B], f16, kind="ExternalInput").ap()
    out_d = nc.dram_tensor("out_sh", [B, N], f32, kind="ExternalOutput").ap()

    with tile.TileContext(nc) as tc, ExitStack() as ctx:
        const = ctx.enter_context(tc.tile_pool(name="const", bufs=1))
        work = ctx.enter_context(tc.tile_pool(name="work", bufs=1))
        psum = ctx.enter_context(tc.tile_pool(name="psum", bufs=1, space="PSUM"))

        # ---- input DMAs on three independent DGE paths ----
        DR = const.tile([P_SH, N], f16)
        W = const.tile([P_SH, N], f16)
        BUF = const.tile([P_SH, D_HI * B], f16)
        nc.sync.dma_start(DR[:], dr_d[:])      # SP HWDGE
        nc.gpsimd.dma_start(BUF[:], buf_d[:])  # Pool SWDGE
        nc.scalar.dma_start(W[:], w_d[:])      # ACT HWDGE

        PSL = psum.tile([B, 512], f32)
        PSR = psum.tile([B, 512], f32)

        NEGI = const.tile([P_SH, max(E, 2)], i32)
        nc.gpsimd.iota(NEGI[:], pattern=[[-1, max(E, 2)]], base=-1, channel_multiplier=0)

        # dep-free work first: PSUM zeroing + bias cast (Pool cannot touch PSUM)
        nc.vector.memset(PSL[:], 0.0)
        nc.vector.memset(PSR[:], 0.0)
        NEGD = const.tile([P_SH, max(E, 2)], f32)
        nc.vector.tensor_copy(NEGD[:], NEGI[:])
        ZF = const.tile([P_SH, 512], f16)
        nc.gpsimd.memset(ZF[:], 0.0)

        # ---- prologue: lhsT prep (needs BUF, lands first), then sigmoid path
        BD1 = const.tile([P_SH, B], f16)
        nc.vector.tensor_sub(BD1[:], BUF[:, B : 2 * B], BUF[:, 0:B])
        TSU = const.tile([P_SH, E * B], f16)
        nc.vector.tensor_add(TSU[:], BUF[:, 0 : E * B], BUF[:, 2 * B : (E + 2) * B])
        C2 = const.tile([P_SH, E * B], f16)
        nc.vector.scalar_tensor_tensor(
            C2[:], BUF[:, B : (E + 1) * B], -2.0, TSU[:], OP.mult, OP.add
        )

        SIG = const.tile([P_SH, N], f16)
        nc.scalar.activation(SIG[:], DR[:], AF.Sigmoid)
        X50 = const.tile([P_SH, N], f16)
        nc.vector.tensor_scalar_mul(X50[:], SIG[:], 50.0)
        U = const.tile([P_SH, N], f16)

        # ---- masks (e ascending = widest first) ----
        S_tiles = [None] * E
        R_tiles = [None] * E
        ar_idx = [i for i in range(E) if routes[i] == "AR"]
        for i in ar_idx:  # ACT queue
            m, e = m_list[i], i + 1
            R_tiles[i] = const.tile([P_SH, m], f16, name=f"R{e}", tag=f"R{e}")
            nc.scalar.activation(
                R_tiles[i][:], SIG[:, 0:m], AF.Relu,
                bias=NEGD[:, i : i + 1], scale=50.0,
            )
        def emit_mask(i):
            m, e = m_list[i], i + 1
            S_tiles[i] = const.tile([P_SH, m], f16, name=f"S{e}", tag=f"S{e}")
            if routes[i] == "AD2":
                R_tiles[i] = const.tile([P_SH, m], f16, name=f"R{e}", tag=f"R{e}")
                nc.vector.tensor_scalar(
                    R_tiles[i][:], X50[:, 0:m], float(-e), 0.0, OP.add, OP.max
                )
            nc.vector.tensor_mul(S_tiles[i][:], R_tiles[i][:], W[:, 0:m])

        for i in range(min(2, E)):  # DVE queue: two masks before U
            emit_mask(i)
        nc.vector.tensor_mul(U[:], X50[:], W[:])
        for i in range(min(2, E), E):
            emit_mask(i)

        # ---- accumulation matmuls in estimated-ready order ----
        def bank_touch(item):
            if isinstance(item, str) and item.startswith("Z"):
                return (False, False)
            if item in ("WL", "UL"):
                return (True, False)
            if item in ("WR", "UR"):
                return (False, True)
            return (True, m_list[item] > 512)

        lastL = max(oi for oi, it in enumerate(pe_order) if bank_touch(it)[0])
        lastR = max(oi for oi, it in enumerate(pe_order) if bank_touch(it)[1])
        for oi, item in enumerate(pe_order):
            stopL, stopR = oi == lastL, oi == lastR
            if isinstance(item, str) and item.startswith("Z"):
                nc.tensor.matmul(PSL[:], BUF[:, 0:B], ZF[:],
                                 start=False, stop=False, skip_group_check=True)
            elif item == "WL":
                nc.tensor.matmul(PSL[:], BUF[:, 0:B], W[:, 0:512],
                                 start=False, stop=stopL, skip_group_check=True)
            elif item == "WR":
                nc.tensor.matmul(PSR[:], BUF[:, 0:B], W[:, 512:N],
                                 start=False, stop=stopR, skip_group_check=True)
            elif item == "UL":
                nc.tensor.matmul(PSL[:], BD1[:], U[:, 0:512],
                                 start=False, stop=stopL, skip_group_check=True)
            elif item == "UR":
                nc.tensor.matmul(PSR[:], BD1[:], U[:, 512:N],
                                 start=False, stop=stopR, skip_group_check=True)
            else:
                i = item
                m = m_list[i]
                lhsT = C2[:, i * B : (i + 1) * B]
                nc.tensor.matmul(PSL[:, 0 : min(m, 512)], lhsT,
                                 S_tiles[i][:, 0 : min(m, 512)],
                                 start=False, stop=stopL, skip_group_check=True)
                if m > 512:
                    nc.tensor.matmul(PSR[:, 0 : m - 512], lhsT, S_tiles[i][:, 512:m],
                                     start=False, stop=stopR, skip_group_check=True)

        # ---- output ----
        # PSR closes once all wide-m work is done; copy + DMA it early so only
        # the PSL half trails the final matmul.
        OUT = work.tile([B, N], f32)
        nc.scalar.mul(OUT[:, 512:N], PSR[:], 1.0)
        nc.sync.dma_start(out_d[:, 512:N], OUT[:, 512:N])
        nc.scalar.mul(OUT[:, 0:512], PSL[:], 1.0)
        nc.sync.dma_start(out_d[:, 0:512], OUT[:, 0:512])

    nc.compile()
    return nc


def _get_program(cfg):
    if cfg not in _PROGRAM_CACHE:
        _PROGRAM_CACHE[cfg] = _build_program(cfg)
    return _PROGRAM_CACHE[cfg]


def run(buf, weight, delay_raw, trace=False):
    """Shard, run on 8 cores, gather. Returns (output, BassKernelResults)."""
    from concourse.bass_utils import run_bass_kernel_spmd

    buf = np.asarray(buf, dtype=np.float32)
    weight = np.asarray(weight, dtype=np.float32)
    delay_raw = np.asarray(delay_raw, dtype=np.float32)
    assert buf.shape == (B, D_FULL, P) and weight.shape == (P, N)

    m_list, perms = _plan(delay_raw)
    d_hi = len(m_list) + 2
    nc = _get_program(m_list)

    in_maps = []
    for k in range(N_CORES):
        p0 = k * P_SH
        perm = perms[k]
        in_maps.append(
            {
                "delay_sh": np.ascontiguousarray(
                    delay_raw[p0 : p0 + P_SH, perm].astype(np.float16)
                ),
                "weight_sh": np.ascontiguousarray(
                    weight[p0 : p0 + P_SH, perm].astype(np.float16)
                ),
                "buf_sh": np.ascontiguousarray(
                    buf[:, 0:d_hi, p0 : p0 + P_SH]
                    .transpose(2, 1, 0)
                    .reshape(P_SH, d_hi * B)
                    .astype(np.float16)
                ),
            }
        )
    res = run_bass_kernel_spmd(nc, in_maps, list(range(N_CORES)), trace=trace)
    out = np.zeros((B, N), dtype=np.float32)
    for k in range(N_CORES):
        out[:, perms[k]] += res.results[k]["out_sh"]
    return out, res


def kernel(buf, weight, delay_raw):
    out, _ = run(buf, weight, delay_raw)
    return out


# revision 12
# speedup vs baseline: 1.0634x; 1.0634x over previous
"""Trainium2 Bass kernel for a delayed-synaptic layer.

Computes, for full inputs
    buf        [B=32, D=51, P=1024]  (circular delay buffer)
    weight     [P, N=1024]
    delay_raw  [P, N]
the output
    I_syn[b, n] = sum_p w[p,n] * ((1-a)*buf[b, df, p] + a*buf[b, df+1, p])
with x = 50*sigmoid(delay_raw), df = floor(x), a = x - df.

Algorithm: summation-by-parts over the hat expansion.  With
hat_d(x) = relu(1-|x-d|) one has s = sum_d buf_d * hat_d(x) and
hat_d = R_{d-1} - 2 R_d + R_{d+1} for R_e(x) = relu(x - e), so

    I = buf_0^T @ W + (buf_1 - buf_0)^T @ U + sum_{e>=1} c_e^T @ (W * R_e)

with U = x*w and c_e = buf_{e-1} - 2 buf_e + buf_{e+1} (the e = -1, 0 terms
are affine since x > 0).  Each R_e mask needs ONE relu pass:
  * AD2 route: DVE tensor_scalar relu (4x fp16) + DVE tensor_mul (2x fp16)
  * AR  route: ACT Relu(50*sig - e)           + DVE tensor_mul (2x fp16)
(The Pool engine does NO elementwise work: it shares SBUF ports with the DVE
and roughly halves DVE throughput while active.)  All data fp16 (downcast on
host); PSUM accumulates fp32.  PSUM banks are zeroed by an early DVE memset
so every matmul carries start=False and the PE queue (strictly in-order) can
be sequenced purely by estimated operand readiness.

Column pruning: core k owns p in [128k, 128k+128).  For its column n,
R_e[:, n] == 0 for all e >= max_p x[p,n] + margin =: h_n.  The host sorts
columns by h_n descending (per-core permutation of weight/delay/output
columns) so step e only needs the first m_e = #{n : h_n > e} columns, a
~1.7x cut in mask + matmul work; excluded columns compute exactly zero.
m_e (max across cores) and the data-derived e-range are baked into the
compiled program; different inputs simply recompile.

Wide-m e's run first: the right PSUM bank (columns 512+) then closes well
before the end, and its copy-out + DMA overlap the remaining small-m work.
Input DMAs use whole tensors (2 KB/partition descriptors) on three
independent DGE queues: BUF via Pool SWDGE, delays via SP, weights via ACT.

Sharding: data-parallel over pre-neurons p (contraction axis); each core
emits a partial [32, 1024] output (own column order); host un-permutes+sums.
"""

import numpy as np

B = 32
D_FULL = 51
P = 1024
N = 1024
N_CORES = 8
P_SH = P // N_CORES  # 128
XMARGIN = 0.25  # host-window safety vs device fp16/table sigmoid

_PROGRAM_CACHE: dict = {}
_LAST_PLAN: dict = {}

# clean-rate estimates (ns/col, measured on HW) -- scheduling only
_TS = 0.33  # DVE tensor_scalar fp16 (4x mode)
_TT = 0.59  # DVE tensor_tensor fp16 (2x mode)
_AC = 1.01  # ACT activation pass
_OH_D, _OH_A = 60.0, 90.0


def _plan(delay_raw):
    """Host planning: e-range, per-core column permutations, shared m_e."""
    x = 50.0 / (1.0 + np.exp(-delay_raw.astype(np.float64)))  # [P, N]
    h_all = x.max() + XMARGIN
    e_hi = max(1, int(np.floor(h_all)))  # R_e == 0 for e >= h_all
    e_hi = min(e_hi, 49)
    perms = []
    m_pc = np.zeros((N_CORES, e_hi + 1), dtype=np.int64)
    for k in range(N_CORES):
        h = x[k * P_SH : (k + 1) * P_SH].max(axis=0) + XMARGIN  # [N]
        perm = np.argsort(-h, kind="stable")
        perms.append(perm)
        hs = h[perm]
        for e in range(1, e_hi + 1):
            m_pc[k, e] = int(np.count_nonzero(hs > e))
    m = m_pc.max(axis=0)
    m_list = []
    for e in range(1, e_hi + 1):
        if m[e] <= 0:
            break
        m_list.append(int(min(N, ((m[e] + 31) // 32) * 32)))
    if not m_list:
        m_list = [64]
    return tuple(m_list), perms


def _sched(m_list):
    """Greedy DVE/ACT route balance + PE order by estimated readiness.
    Processing order is e ascending (widest masks first) so the right PSUM
    bank closes early.  Returns (routes, pe_order) with pseudo-items
    "WL","WR","UL","UR" included in pe_order."""
    E = len(m_list)
    # relative ns offsets: DVE mask stream starts ~0 (X50 done), ACT relus
    # start ~+200 (SIG done); U done ~+700 on the DVE queue.
    tD, tA = 0.0, 600.0
    ready = {}
    routes = {}
    for i in range(E):
        m = m_list[i]
        if i < 2:  # DVE-only starters while ACT finishes the sigmoid
            routes[i] = "AD2"
            tD = tD + (_TS + _TT) * m + 2 * _OH_D
            ready[i] = tD
            continue
        cD = tD + (_TS + _TT) * m + 2 * _OH_D
        relu_done = tA + _AC * m + _OH_A
        cA = max(relu_done, tD) + _TT * m + _OH_D
        if cD <= cA:
            routes[i] = "AD2"
            tD = cD
            ready[i] = cD
        else:
            routes[i] = "AR"
            tA = relu_done
            tD = max(tD, relu_done) + _TT * m + _OH_D
            ready[i] = tD
    ready["WL"], ready["WR"] = -200.0, -100.0
    u_done = min(ready[i] for i in range(min(2, E))) if E else 0.0
    ready["UL"], ready["UR"] = u_done + 600.0, u_done + 610.0
    pe_order = sorted(ready, key=lambda k: (ready[k], str(k)))
    return routes, pe_order


def _build_program(cfg):
    """Build the (SPMD, identical-per-core) Bass program once per config."""
    from contextlib import ExitStack

    import concourse.tile as tile
    from concourse import bacc, mybir

    m_list = list(cfg)
    E = len(m_list)  # masks for e = 1..E
    D_HI = E + 2  # buf slices used: [0, D_HI); c_e needs buf_{e+1}

    f32 = mybir.dt.float32
    f16 = mybir.dt.float16
    i32 = mybir.dt.int32
    AF = mybir.ActivationFunctionType
    OP = mybir.AluOpType

    routes, pe_order = _sched(m_list)
    _LAST_PLAN.update(m_list=m_list, routes=routes, pe_order=pe_order)

    nc = bacc.Bacc(trn_type="TRN2", target_bir_lowering=False, debug=False)

    dr_d = nc.dram_tensor("delay_sh", [P_SH, N], f16, kind="ExternalInput").ap()
    w_d = nc.dram_tensor("weight_sh", [P_SH, N], f16, kind="ExternalInput").ap()
    buf_d = nc.dram_tensor("buf_sh", [P_SH, D_HI * B], f16, kind="ExternalInput").ap()
    out_d = nc.dram_tensor("out_sh", [B, N], f32, kind="ExternalOutput").ap()

    with tile.TileContext(nc) as tc, ExitStack() as ctx:
        const = ctx.enter_context(tc.tile_pool(name="const", bufs=1))
        work = ctx.enter_context(tc.tile_pool(name="work", bufs=1))
        psum = ctx.enter_context(tc.tile_pool(name="psum", bufs=1, space="PSUM"))

        # ---- input DMAs on three independent DGE paths ----
        DR = const.tile([P_SH, N], f16)
        W = const.tile([P_SH, N], f16)
        BUF = const.tile([P_SH, D_HI * B], f16)
        nc.sync.dma_start(DR[:], dr_d[:])      # SP HWDGE
        nc.gpsimd.dma_start(BUF[:], buf_d[:])  # Pool SWDGE
        nc.scalar.dma_start(W[:], w_d[:])      # ACT HWDGE

        PSL = psum.tile([B, 512], f32)
        PSR = psum.tile([B, 512], f32)

        NEGI = const.tile([P_SH, max(E, 2)], i32)
        nc.gpsimd.iota(NEGI[:], pattern=[[-1, max(E, 2)]], base=-1, channel_multiplier=0)

        # dep-free work first: PSUM zeroing + bias cast (Pool cannot touch PSUM)
        nc.vector.memset(PSL[:], 0.0)
        nc.vector.memset(PSR[:], 0.0)
        NEGD = const.tile([P_SH, max(E, 2)], f32)
        nc.vector.tensor_copy(NEGD[:], NEGI[:])

        # ---- prologue: lhsT prep (needs BUF, lands first), then sigmoid path
        BD1 = const.tile([P_SH, B], f16)
        nc.vector.tensor_sub(BD1[:], BUF[:, B : 2 * B], BUF[:, 0:B])
        TSU = const.tile([P_SH, E * B], f16)
        nc.vector.tensor_add(TSU[:], BUF[:, 0 : E * B], BUF[:, 2 * B : (E + 2) * B])
        C2 = const.tile([P_SH, E * B], f16)
        nc.vector.scalar_tensor_tensor(
            C2[:], BUF[:, B : (E + 1) * B], -2.0, TSU[:], OP.mult, OP.add
        )

        SIG = const.tile([P_SH, N], f16)
        nc.scalar.activation(SIG[:], DR[:], AF.Sigmoid)
        X50 = const.tile([P_SH, N], f16)
        nc.vector.tensor_scalar_mul(X50[:], SIG[:], 50.0)
        U = const.tile([P_SH, N], f16)

        # ---- masks (e ascending = widest first) ----
        S_tiles = [None] * E
        R_tiles = [None] * E
        ar_idx = [i for i in range(E) if routes[i] == "AR"]
        for i in ar_idx:  # ACT queue
            m, e = m_list[i], i + 1
            R_tiles[i] = const.tile([P_SH, m], f16, name=f"R{e}", tag=f"R{e}")
            nc.scalar.activation(
                R_tiles[i][:], SIG[:, 0:m], AF.Relu,
                bias=NEGD[:, i : i + 1], scale=50.0,
            )
        def emit_mask(i):
            m, e = m_list[i], i + 1
            S_tiles[i] = const.tile([P_SH, m], f16, name=f"S{e}", tag=f"S{e}")
            if routes[i] == "AD2":
                R_tiles[i] = const.tile([P_SH, m], f16, name=f"R{e}", tag=f"R{e}")
                nc.vector.tensor_scalar(
                    R_tiles[i][:], X50[:, 0:m], float(-e), 0.0, OP.add, OP.max
                )
            nc.vector.tensor_mul(S_tiles[i][:], R_tiles[i][:], W[:, 0:m])

        for i in range(min(2, E)):  # DVE queue: two masks before U
            emit_mask(i)
        nc.vector.tensor_mul(U[:], X50[:], W[:])
        for i in range(min(2, E), E):
            emit_mask(i)

        # ---- accumulation matmuls in estimated-ready order ----
        def bank_touch(item):
            if item in ("WL", "UL"):
                return (True, False)
            if item in ("WR", "UR"):
                return (False, True)
            return (True, m_list[item] > 512)

        lastL = max(oi for oi, it in enumerate(pe_order) if bank_touch(it)[0])
        lastR = max(oi for oi, it in enumerate(pe_order) if bank_touch(it)[1])
        for oi, item in enumerate(pe_order):
            stopL, stopR = oi == lastL, oi == lastR
            if item == "WL":
                nc.tensor.matmul(PSL[:], BUF[:, 0:B], W[:, 0:512],
                                 start=False, stop=stopL, skip_group_check=True)
            elif item == "WR":
                nc.tensor.matmul(PSR[:], BUF[:, 0:B], W[:, 512:N],
                                 start=False, stop=stopR, skip_group_check=True)
            elif item == "UL":
                nc.tensor.matmul(PSL[:], BD1[:], U[:, 0:512],
                                 start=False, stop=stopL, skip_group_check=True)
            elif item == "UR":
                nc.tensor.matmul(PSR[:], BD1[:], U[:, 512:N],
                                 start=False, stop=stopR, skip_group_check=True)
            else:
                i = item
                m = m_list[i]
                lhsT = C2[:, i * B : (i + 1) * B]
                nc.tensor.matmul(PSL[:, 0 : min(m, 512)], lhsT,
                                 S_tiles[i][:, 0 : min(m, 512)],
                                 start=False, stop=stopL, skip_group_check=True)
                if m > 512:
                    nc.tensor.matmul(PSR[:, 0 : m - 512], lhsT, S_tiles[i][:, 512:m],
                                     start=False, stop=stopR, skip_group_check=True)

        # ---- output ----
        # PSR closes once all wide-m work is done; copy + DMA it early so only
        # the PSL half trails the final matmul.
        OUT = work.tile([B, N], f32)
        nc.scalar.mul(OUT[:, 512:N], PSR[:], 1.0)
        nc.sync.dma_start(out_d[:, 512:N], OUT[:, 512:N])
        nc.scalar.mul(OUT[:, 0:512], PSL[:], 1.0)
        nc.sync.dma_start(out_d[:, 0:512], OUT[:, 0:512])

    nc.compile()
    return nc


def _get_program(cfg):
    if cfg not in _PROGRAM_CACHE:
        _PROGRAM_CACHE[cfg] = _build_program(cfg)
    return _PROGRAM_CACHE[cfg]


def run(buf, weight, delay_raw, trace=False):
    """Shard, run on 8 cores, gather. Returns (output, BassKernelResults)."""
    from concourse.bass_utils import run_bass_kernel_spmd

    buf = np.asarray(buf, dtype=np.float32)
    weight = np.asarray(weight, dtype=np.float32)
    delay_raw = np.asarray(delay_raw, dtype=np.float32)
    assert buf.shape == (B, D_FULL, P) and weight.shape == (P, N)

    m_list, perms = _plan(delay_raw)
    d_hi = len(m_list) + 2
    nc = _get_program(m_list)

    in_maps = []
    for k in range(N_CORES):
        p0 = k * P_SH
        perm = perms[k]
        in_maps.append(
            {
                "delay_sh": np.ascontiguousarray(
                    delay_raw[p0 : p0 + P_SH, perm].astype(np.float16)
                ),
                "weight_sh": np.ascontiguousarray(
                    weight[p0 : p0 + P_SH, perm].astype(np.float16)
                ),
                "buf_sh": np.ascontiguousarray(
                    buf[:, 0:d_hi, p0 : p0 + P_SH]
                    .transpose(2, 1, 0)
                    .reshape(P_SH, d_hi * B)
                    .astype(np.float16)
                ),
            }
        )
    res = run_bass_kernel_spmd(nc, in_maps, list(range(N_CORES)), trace=trace)
    out = np.zeros((B, N), dtype=np.float32)
    for k in range(N_CORES):
        out[:, perms[k]] += res.results[k]["out_sh"]
    return out, res


def kernel(buf, weight, delay_raw):
    out, _ = run(buf, weight, delay_raw)
    return out
